# revision 5
# baseline (speedup 1.0000x reference)
"""NaViT packed-sequence ViT forward on 8 Trainium2 NeuronCores.

Sharding: the packed groups have block-diagonal attention (4 images x 256
tokens per group, verified at runtime from image_ids/lengths). The network
then decomposes per image -> 16 independent images, 2 per core, T=512
tokens per core, zero cross-core communication.

Per-core kernel layout:
  - residual stream x: token-major [128 part, 4 tiles, 768] fp32
  - matmul operands bf16; LN/softmax/statistics fp32
  - dense matmuls token-major (lhsT = PE-transposed activations)
  - attention per (image, head) with scores^T [j, i]; softmax denominator
    via an extra ones-column appended to V (no max-subtraction needed:
    |scores| <= 64*|gq*gk| after qk-rmsnorm, safely inside fp32 exp range)
"""

import sys

sys.path.insert(0, "/opt/trn_rl_repo")

import numpy as np
import ml_dtypes

B, IMGS, PH, PW = 4, 4, 16, 16
P, C = 16, 3
N = IMGS * PH * PW            # 1024
PATCH_DIM = C * P * P         # 768
DIM, HEADS, DHEAD, DEPTH = 768, 12, 64, 4
INNER = HEADS * DHEAD         # 768
MLP = 3072
NCLS = 1000
SCALE = DHEAD ** 0.5

NCORES = 8
IMG_PER_CORE = 2
T = IMG_PER_CORE * PH * PW    # 512 tokens per core
TT = T // 128                 # 4 token tiles
KD = DIM // 128               # 6 feature chunks
MC = MLP // 128               # 24 mlp chunks
TOK_IMG = PH * PW             # 256


def _ln_np(x, g, eps=1e-5):
    mu = x.mean(-1, keepdims=True)
    var = x.var(-1, keepdims=True)
    return (x - mu) / np.sqrt(var + eps) * g


def _rms_np(t, g):
    nrm = np.sqrt((t * t).sum(-1, keepdims=True))
    return t / np.maximum(nrm, 1e-12) * SCALE * g


def _softmax_np(x, axis):
    m = x.max(axis=axis, keepdims=True)
    e = np.exp(x - m)
    return e / e.sum(axis=axis, keepdims=True)


def _erf(x):
    # Abramowitz-Stegun style is too inaccurate; use tanh-free exact via
    # scipy if present, else vectorized math.erf.
    try:
        from scipy.special import erf as _serf
        return _serf(x)
    except Exception:
        import math
        return np.vectorize(math.erf)(x).astype(x.dtype)


def _reference_np(**inp):
    """Numpy port of the oracle; only used for non-block-diagonal inputs."""
    f32 = np.float32
    patches = inp["patches"].astype(f32)
    image_ids = np.asarray(inp["image_ids"])
    lengths = np.asarray(inp["lengths"])
    valid = np.arange(N)[None, :] < lengths[:, None]
    same = image_ids[:, :, None] == image_ids[:, None, :]
    attn_mask = (same & valid[:, None, :])[:, None]

    x = _ln_np(patches, inp["emb_ln_g"]) @ inp["W_emb"] + inp["b_emb"]
    x = _ln_np(x, inp["emb_ln2_g"])
    pp = np.asarray(inp["patch_positions"])
    x = x + inp["pos_h"][pp[..., 0]] + inp["pos_w"][pp[..., 1]]

    def attention(x_, context, ln_g, Wq, Wkv, qg, kg, Wo, mask):
        xn = _ln_np(x_, ln_g)
        kv_in = xn if context is None else context
        q = xn @ Wq
        k, v = np.split(kv_in @ Wkv, 2, axis=-1)

        def split(t):
            return t.reshape(t.shape[0], t.shape[1], HEADS, DHEAD).transpose(0, 2, 1, 3)

        q, k, v = split(q), split(k), split(v)
        q = _rms_np(q, qg[:, None, :])
        k = _rms_np(k, kg[:, None, :])
        dots = np.einsum("bhid,bhjd->bhij", q, k)
        dots = np.where(mask, dots, -np.finfo(f32).max)
        attn = _softmax_np(dots, -1)
        out = np.einsum("bhij,bhjd->bhid", attn, v)
        out = out.transpose(0, 2, 1, 3).reshape(x_.shape[0], -1, INNER)
        return out @ Wo

    for l in range(DEPTH):
        x = attention(x, None, inp["ln_attn_g"][l], inp["Wq"][l], inp["Wkv"][l],
                      inp["qn_g"][l], inp["kn_g"][l], inp["Wo"][l], attn_mask) + x
        h = _ln_np(x, inp["ln_ff_g"][l]) @ inp["W1"][l] + inp["b1"][l]
        h = h * 0.5 * (1.0 + _erf(h / np.sqrt(2.0)))
        x = h @ inp["W2"][l] + inp["b2"][l] + x
    x = _ln_np(x, inp["final_ln_g"])

    queries = np.broadcast_to(inp["pool_q"], (B, IMGS, DIM))
    pool_mask = ((np.arange(IMGS)[None, :, None] == image_ids[:, None, :])
                 & valid[:, None, :])[:, None]
    pooled = attention(queries, x, inp["pool_ln_g"], inp["pWq"], inp["pWkv"],
                       inp["p_qn_g"], inp["p_kn_g"], inp["pWo"], pool_mask) + queries
    return (_ln_np(pooled, inp["head_ln_g"]) @ inp["W_head"]).astype(f32)


# ---------------------------------------------------------------------------
# Bass kernel
# ---------------------------------------------------------------------------

_CACHE = {}


def build_kernel():
    import concourse.bass as bass
    from concourse import bacc
    import concourse.mybir as mybir
    import concourse.tile as tile
    from concourse.masks import make_identity

    F32 = mybir.dt.float32
    BF16 = mybir.dt.bfloat16
    AF = mybir.ActivationFunctionType
    ALU = mybir.AluOpType
    AX = mybir.AxisListType

    nc = bacc.Bacc()

    def din(name, shape, dt=F32):
        return nc.declare_dram_parameter(name, list(shape), dt, isOutput=False)

    patches_d = din("patches", [T, PATCH_DIM])
    pos_d = din("pos_add", [T, DIM])
    W_emb_d = din("W_emb", [PATCH_DIM, DIM], BF16)
    Wq_d = din("Wq", [DEPTH, DIM, INNER], BF16)
    Wkv_d = din("Wkv", [DEPTH, DIM, 2 * INNER], BF16)
    Wo_d = din("Wo", [DEPTH, INNER, DIM], BF16)
    W1_d = din("W1", [DEPTH, DIM, MLP], BF16)
    W2_d = din("W2", [DEPTH, MLP, DIM], BF16)
    pWkv_d = din("pWkv", [DIM, 2 * INNER], BF16)
    pWo_d = din("pWo", [INNER, DIM], BF16)
    W_head_d = din("W_head", [DIM, NCLS], BF16)
    emb_ln_g_d = din("emb_ln_g", [PATCH_DIM])
    emb_ln2_g_d = din("emb_ln2_g", [DIM])
    b_emb_d = din("b_emb", [DIM])
    ln_attn_g_d = din("ln_attn_g", [DEPTH, DIM])
    ln_ff_g_d = din("ln_ff_g", [DEPTH, DIM])
    qg_row_d = din("qg_row", [DEPTH, INNER])     # qn_g * SCALE
    kg_row_d = din("kg_row", [DEPTH, INNER])
    b1_d = din("b1", [DEPTH, MLP])
    b2_d = din("b2", [DEPTH, DIM])
    final_ln_g_d = din("final_ln_g", [DIM])
    pk_row_d = din("pk_row", [INNER])            # p_kn_g * SCALE
    qpool_d = din("qpool", [128, HEADS], BF16)   # rows 0:64 == rows 64:128
    pool_q_d = din("pool_q", [DIM])
    head_ln_g_d = din("head_ln_g", [DIM])
    out_d = nc.declare_dram_parameter("out", [IMG_PER_CORE, NCLS], F32,
                                      isOutput=True)

    def bcast_ap(dram, row, width, parts=128):
        ap = dram.ap()
        off = ap.offset + (0 if row is None else row * width)
        return bass.AP(tensor=ap.tensor, offset=off, ap=[[0, parts], [1, width]])

    with tile.TileContext(nc) as tc:
        with (
            tc.tile_pool(name="const", bufs=1) as constp,
            tc.tile_pool(name="resid", bufs=1) as residp,
            tc.tile_pool(name="act", bufs=1) as actp,
            tc.tile_pool(name="wts", bufs=3) as wpool,
            tc.tile_pool(name="aux", bufs=2) as auxp,
            tc.tile_pool(name="small", bufs=2) as smallp,
            tc.tile_pool(name="attn", bufs=3) as attnp,
            tc.tile_pool(name="ps", bufs=2, space="PSUM") as psp,
        ):
            ident = constp.tile([128, 128], BF16)
            make_identity(nc, ident)
            eps_t = constp.tile([128, 1], F32)
            nc.vector.memset(eps_t, 1e-5)

            x_t = residp.tile([128, TT, DIM], F32, tag="x")
            xn_t = residp.tile([128, TT, DIM], BF16, tag="xn")
            xnT_t = residp.tile([128, KD, T], BF16, tag="xnT")
            q_t = residp.tile([128, TT, INNER], BF16, tag="q")
            k_t = residp.tile([128, TT, INNER], BF16, tag="k")
            v_t = residp.tile([128, TT, HEADS, DHEAD + 1], BF16, tag="v")
            qT_t = residp.tile([128, KD, T], BF16, tag="qT")
            kT_t = residp.tile([128, KD, T], BF16, tag="kT")
            av_t = q_t      # q is dead once qT exists; reuse for attn output
            avT_t = residp.tile([128, KD, T], BF16, tag="avT")
            h_t = residp.tile([128, MC // 2, T], BF16, tag="h")

            nc.vector.memset(v_t[:, :, :, DHEAD:DHEAD + 1], 1.0)

            # ---------------- helpers ----------------
            def ln_to(dst, src_f32, gamma_bc):
                """dst = LN(src) * gamma_bc, token-major [128,TT,DIM]."""
                for it in range(TT):
                    st = smallp.tile([128, 3, nc.vector.BN_STATS_DIM], F32, tag="st")
                    xr = src_f32[:, it, :].rearrange("p (n f) -> p n f", f=256)
                    for i in range(3):
                        nc.vector.bn_stats(out=st[:, i, :], in_=xr[:, i, :])
                    mv = smallp.tile([128, nc.vector.BN_AGGR_DIM], F32, tag="mv")
                    nc.vector.bn_aggr(out=mv, in_=st)
                    rstd = smallp.tile([128, 1], F32, tag="rstd")
                    nc.scalar.activation(out=rstd, in_=mv[:, 1:2], func=AF.Sqrt,
                                         bias=eps_t, scale=1.0)
                    nc.vector.reciprocal(out=rstd, in_=rstd)
                    tmp = smallp.tile([128, DIM], F32, tag="lntmp")
                    nc.vector.scalar_tensor_tensor(
                        out=tmp, in0=src_f32[:, it, :], scalar=mv[:, 0:1],
                        in1=gamma_bc, op0=ALU.subtract, op1=ALU.mult)
                    nc.vector.tensor_scalar_mul(out=dst[:, it, :], in0=tmp,
                                                scalar1=rstd)

            def transpose_tm(dst_fm, src_tm, ncol=DIM):
                """token-major [128,TT,ncol] bf16 -> feature-major [128,nc,T]."""
                for c in range(ncol // 128):
                    ps = psp.tile([128, T], BF16, tag="tp")
                    for it in range(TT):
                        nc.tensor.transpose(
                            ps[:, it * 128:(it + 1) * 128],
                            src_tm[:, it, c * 128:(c + 1) * 128], ident)
                    nc.vector.tensor_copy(out=dst_fm[:, c, :], in_=ps)

            def load_w(dram_l, rows, cols, tag="wt", row_off=0, col_off=0,
                       bufs=None):
                wt = wpool.tile([128, rows // 128, cols], BF16, tag=tag,
                                **({} if bufs is None else {"bufs": bufs}))
                src = dram_l.rearrange("(c p) n -> p c n", p=128)
                nc.sync.dma_start(
                    out=wt, in_=src[:, row_off // 128:(row_off + rows) // 128,
                                    col_off:col_off + cols])
                return wt

            def mm_tok(dst_tm, lhsT_fm, w_tile, ncol, col_base=0, add_resid=None,
                       nk=KD):
                """token-major matmul: dst[:, it, g] = lhsT_fm.T @ w[:, col_base+g]."""
                for it in range(TT):
                    for g0 in range(0, ncol, 512):
                        gw = min(512, ncol - g0)
                        ps = psp.tile([128, 512], F32, tag="mm")
                        for c in range(nk):
                            nc.tensor.matmul(
                                ps[:, :gw],
                                lhsT_fm[:, c, it * 128:(it + 1) * 128],
                                w_tile[:, c, col_base + g0:col_base + g0 + gw],
                                start=(c == 0), stop=(c == nk - 1))
                        if add_resid is not None:
                            nc.vector.tensor_add(
                                out=add_resid[:, it, g0:g0 + gw],
                                in0=add_resid[:, it, g0:g0 + gw], in1=ps[:, :gw])
                        else:
                            nc.vector.tensor_copy(out=dst_tm[:, it, g0:g0 + gw],
                                                  in_=ps[:, :gw])

            def mm_v(lhsT_fm, w_tile, col_base):
                """like mm_tok but scatters per-head into v_t's 65-stride slots."""
                for it in range(TT):
                    for g0 in range(0, INNER, 512):
                        gw = min(512, INNER - g0)
                        ps = psp.tile([128, 512], F32, tag="mm")
                        for c in range(KD):
                            nc.tensor.matmul(
                                ps[:, :gw],
                                lhsT_fm[:, c, it * 128:(it + 1) * 128],
                                w_tile[:, c, col_base + g0:col_base + g0 + gw],
                                start=(c == 0), stop=(c == KD - 1))
                        for hh in range(g0 // DHEAD, (g0 + gw) // DHEAD):
                            nc.vector.tensor_copy(
                                out=v_t[:, it, hh, 0:DHEAD],
                                in_=ps[:, hh * DHEAD - g0:(hh + 1) * DHEAD - g0])

            def rmsnorm_tok(t_tm, g_bc):
                for it in range(TT):
                    sq = smallp.tile([128, INNER], F32, tag="sq")
                    nc.vector.tensor_mul(out=sq, in0=t_tm[:, it, :],
                                         in1=t_tm[:, it, :])
                    ss = smallp.tile([128, HEADS], F32, tag="ss")
                    nc.vector.tensor_reduce(
                        out=ss, in_=sq.rearrange("p (h d) -> p h d", d=DHEAD),
                        axis=AX.X, op=ALU.add)
                    nrm = smallp.tile([128, HEADS], F32, tag="nrm")
                    nc.scalar.activation(out=nrm, in_=ss, func=AF.Sqrt, scale=1.0)
                    nc.vector.tensor_scalar_max(out=nrm, in0=nrm, scalar1=1e-12)
                    nc.vector.reciprocal(out=nrm, in_=nrm)
                    for hh in range(HEADS):
                        nc.vector.tensor_scalar_mul(
                            out=t_tm[:, it, hh * DHEAD:(hh + 1) * DHEAD],
                            in0=t_tm[:, it, hh * DHEAD:(hh + 1) * DHEAD],
                            scalar1=nrm[:, hh:hh + 1])
                    nc.vector.tensor_mul(out=t_tm[:, it, :], in0=t_tm[:, it, :],
                                         in1=g_bc)

            def load_aux(dram, row, width, tag):
                t = auxp.tile([128, width], F32, tag=tag)
                nc.sync.dma_start(out=t, in_=bcast_ap(dram, row, width))
                return t

            # ---------------- embedding ----------------
            pt = wpool.tile([128, TT, PATCH_DIM], F32, tag="wt")
            nc.sync.dma_start(out=pt,
                              in_=patches_d.ap().rearrange("(t p) d -> p t d",
                                                           p=128))
            g_emb = load_aux(emb_ln_g_d, None, PATCH_DIM, "g1")
            ln_to(xn_t, pt, g_emb)
            transpose_tm(xnT_t, xn_t)
            wemb = load_w(W_emb_d.ap(), PATCH_DIM, DIM)
            b_emb_bc = load_aux(b_emb_d, None, DIM, "g2")
            for it in range(TT):
                for g0 in range(0, DIM, 512):
                    gw = min(512, DIM - g0)
                    ps = psp.tile([128, 512], F32, tag="mm")
                    for c in range(KD):
                        nc.tensor.matmul(ps[:, :gw],
                                         xnT_t[:, c, it * 128:(it + 1) * 128],
                                         wemb[:, c, g0:g0 + gw],
                                         start=(c == 0), stop=(c == KD - 1))
                    nc.vector.tensor_add(out=x_t[:, it, g0:g0 + gw],
                                         in0=ps[:, :gw],
                                         in1=b_emb_bc[:, g0:g0 + gw])
            g_emb2 = load_aux(emb_ln2_g_d, None, DIM, "g1")
            ln_to(x_t, x_t, g_emb2)
            post = wpool.tile([128, TT, DIM], F32, tag="wt")
            nc.sync.dma_start(out=post,
                              in_=pos_d.ap().rearrange("(t p) d -> p t d", p=128))
            for it in range(TT):
                nc.vector.tensor_add(out=x_t[:, it, :], in0=x_t[:, it, :],
                                     in1=post[:, it, :])

            # ---------------- transformer layers ----------------
            for l in range(DEPTH):
                g1 = load_aux(ln_attn_g_d, l, DIM, "g1")
                ln_to(xn_t, x_t, g1)
                transpose_tm(xnT_t, xn_t)

                wq = load_w(Wq_d[l], DIM, INNER)
                mm_tok(q_t, xnT_t, wq, INNER)
                wkv = load_w(Wkv_d[l], DIM, 2 * INNER)
                mm_tok(k_t, xnT_t, wkv, INNER)
                mm_v(xnT_t, wkv, INNER)

                gq = load_aux(qg_row_d, l, INNER, "g2")
                rmsnorm_tok(q_t, gq)
                transpose_tm(qT_t, q_t, INNER)
                gk = load_aux(kg_row_d, l, INNER, "g2")
                rmsnorm_tok(k_t, gk)
                transpose_tm(kT_t, k_t, INNER)

                for img in range(IMG_PER_CORE):
                    i0 = img * TOK_IMG
                    for hh in range(HEADS):
                        c, b = hh // 2, (hh % 2) * DHEAD
                        at = attnp.tile([128, 2, TOK_IMG], BF16, tag="at")
                        for jc in range(2):
                            sps = psp.tile([128, TOK_IMG], F32, tag="sc")
                            nc.tensor.matmul(
                                sps,
                                kT_t[b:b + DHEAD, c,
                                     i0 + jc * 128:i0 + (jc + 1) * 128],
                                qT_t[b:b + DHEAD, c, i0:i0 + TOK_IMG],
                                start=True, stop=True)
                            nc.scalar.activation(out=at[:, jc, :], in_=sps,
                                                 func=AF.Exp)
                        for ic in range(2):
                            aps = psp.tile([128, DHEAD + 1], F32, tag="av")
                            for jc in range(2):
                                nc.tensor.matmul(
                                    aps,
                                    at[:, jc, ic * 128:(ic + 1) * 128],
                                    v_t[:, 2 * img + jc, hh, :],
                                    start=(jc == 0), stop=(jc == 1))
                            rs = smallp.tile([128, 1], F32, tag="rs")
                            nc.vector.reciprocal(out=rs,
                                                 in_=aps[:, DHEAD:DHEAD + 1])
                            nc.vector.tensor_scalar_mul(
                                out=av_t[:, 2 * img + ic,
                                         hh * DHEAD:(hh + 1) * DHEAD],
                                in0=aps[:, 0:DHEAD], scalar1=rs)

                transpose_tm(avT_t, av_t, INNER)
                wo = load_w(Wo_d[l], INNER, DIM)
                mm_tok(None, avT_t, wo, DIM, add_resid=x_t)

                # MLP
                g2t = load_aux(ln_ff_g_d, l, DIM, "g1")
                ln_to(xn_t, x_t, g2t)
                transpose_tm(xnT_t, xn_t)
                b1t = auxp.tile([128, MC], F32, tag="b1")
                nc.sync.dma_start(out=b1t,
                                  in_=b1_d[l].rearrange("(c p) -> p c", p=128))
                for half in range(2):
                    w1 = load_w(W1_d[l], DIM, MLP // 2, col_off=half * (MLP // 2))
                    for cm0 in range(MC // 2):
                        cm = half * (MC // 2) + cm0
                        ps = psp.tile([128, T], F32, tag="mm")
                        for c in range(KD):
                            nc.tensor.matmul(ps,
                                             w1[:, c, cm0 * 128:(cm0 + 1) * 128],
                                             xnT_t[:, c, :],
                                             start=(c == 0), stop=(c == KD - 1))
                        nc.scalar.activation(out=h_t[:, cm0, :], in_=ps,
                                             func=AF.Gelu,
                                             bias=b1t[:, cm:cm + 1], scale=1.0)
                    w2 = load_w(W2_d[l], MLP // 2, DIM, row_off=half * (MLP // 2))
                    for it in range(TT):
                        for g0 in range(0, DIM, 512):
                            gw = min(512, DIM - g0)
                            ps = psp.tile([128, 512], F32, tag="mm")
                            for cm0 in range(MC // 2):
                                nc.tensor.matmul(
                                    ps[:, :gw],
                                    h_t[:, cm0, it * 128:(it + 1) * 128],
                                    w2[:, cm0, g0:g0 + gw],
                                    start=(cm0 == 0), stop=(cm0 == MC // 2 - 1))
                            nc.vector.tensor_add(
                                out=x_t[:, it, g0:g0 + gw],
                                in0=x_t[:, it, g0:g0 + gw], in1=ps[:, :gw])
                b2bc = load_aux(b2_d, l, DIM, "g2")
                for it in range(TT):
                    nc.vector.tensor_add(out=x_t[:, it, :], in0=x_t[:, it, :],
                                         in1=b2bc)

            # ---------------- final LN + attention pooling ----------------
            gf = load_aux(final_ln_g_d, None, DIM, "g1")
            ln_to(xn_t, x_t, gf)
            transpose_tm(xnT_t, xn_t)

            pwkv = load_w(pWkv_d.ap(), DIM, 2 * INNER)
            mm_tok(k_t, xnT_t, pwkv, INNER)
            mm_v(xnT_t, pwkv, INNER)
            gpk = load_aux(pk_row_d, None, INNER, "g2")
            rmsnorm_tok(k_t, gpk)
            transpose_tm(kT_t, k_t, INNER)

            qpool = constp.tile([128, HEADS], BF16)
            nc.sync.dma_start(out=qpool, in_=qpool_d[:, :])

            def transpose_1row(src_bf, tag):
                """[1, DIM] bf16 -> [128, KD, 1] bf16."""
                dst = actp.tile([128, KD, 1], BF16, tag=tag)
                ps = psp.tile([128, KD, 2], BF16, tag="tp")
                for c in range(KD):
                    nc.tensor.transpose(ps[:, c, 0:1],
                                        src_bf[0:1, c * 128:(c + 1) * 128],
                                        ident[0:1, 0:1])
                nc.vector.tensor_copy(out=dst, in_=ps[:, :, 0:1])
                return dst

            pwo = load_w(pWo_d.ap(), INNER, DIM)
            pq_bc = load_aux(pool_q_d, None, DIM, "g2")
            ghead = load_aux(head_ln_g_d, None, DIM, "g1")
            whead = load_w(W_head_d.ap(), DIM, NCLS)

            for img in range(IMG_PER_CORE):
                i0 = img * TOK_IMG
                pooled = actp.tile([1, DIM], F32, tag="pooled")
                for hh in range(HEADS):
                    c, b = hh // 2, (hh % 2) * DHEAD
                    acl = attnp.tile([128, 2], BF16, tag="acl")
                    for jc in range(2):
                        sps = psp.tile([128, 1], F32, tag="sc")
                        nc.tensor.matmul(
                            sps,
                            kT_t[b:b + DHEAD, c,
                                 i0 + jc * 128:i0 + (jc + 1) * 128],
                            qpool[b:b + DHEAD, hh:hh + 1],
                            start=True, stop=True)
                        nc.scalar.activation(out=acl[:, jc:jc + 1], in_=sps,
                                             func=AF.Exp)
                    aps = psp.tile([128, DHEAD + 1], F32, tag="av")
                    for jc in range(2):
                        nc.tensor.matmul(
                            aps[0:1, :],
                            acl[:, jc:jc + 1],
                            v_t[:, 2 * img + jc, hh, :],
                            start=(jc == 0), stop=(jc == 1))
                    rs = smallp.tile([1, 1], F32, tag="rsp")
                    nc.vector.reciprocal(out=rs, in_=aps[0:1, DHEAD:DHEAD + 1])
                    nc.vector.tensor_scalar_mul(
                        out=pooled[0:1, hh * DHEAD:(hh + 1) * DHEAD],
                        in0=aps[0:1, 0:DHEAD], scalar1=rs)

                pooled_bf = actp.tile([1, DIM], BF16, tag="pooledb")
                nc.vector.tensor_copy(out=pooled_bf, in_=pooled)
                pT = transpose_1row(pooled_bf, "p2T")
                pool2 = actp.tile([1, DIM], F32, tag="pool2")
                for g0 in range(0, DIM, 512):
                    gw = min(512, DIM - g0)
                    ps = psp.tile([128, 512], F32, tag="mm")
                    for c in range(KD):
                        nc.tensor.matmul(ps[0:1, :gw], pT[:, c, :],
                                         pwo[:, c, g0:g0 + gw],
                                         start=(c == 0), stop=(c == KD - 1))
                    nc.vector.tensor_add(out=pool2[:, g0:g0 + gw],
                                         in0=ps[0:1, :gw],
                                         in1=pq_bc[0:1, g0:g0 + gw])

                st = smallp.tile([1, 3, nc.vector.BN_STATS_DIM], F32, tag="stp")
                pr = pool2.rearrange("p (n f) -> p n f", f=256)
                for i in range(3):
                    nc.vector.bn_stats(out=st[:, i, :], in_=pr[:, i, :])
                mv = smallp.tile([1, nc.vector.BN_AGGR_DIM], F32, tag="mvp")
                nc.vector.bn_aggr(out=mv, in_=st)
                rstd = smallp.tile([1, 1], F32, tag="rstdp")
                nc.scalar.activation(out=rstd, in_=mv[:, 1:2], func=AF.Sqrt,
                                     bias=eps_t[0:1], scale=1.0)
                nc.vector.reciprocal(out=rstd, in_=rstd)
                hn = actp.tile([1, DIM], BF16, tag="hn")
                tmp = smallp.tile([1, DIM], F32, tag="hntmp")
                nc.vector.scalar_tensor_tensor(out=tmp, in0=pool2,
                                               scalar=mv[:, 0:1],
                                               in1=ghead[0:1, :],
                                               op0=ALU.subtract, op1=ALU.mult)
                nc.vector.tensor_scalar_mul(out=hn, in0=tmp, scalar1=rstd)

                hT = transpose_1row(hn, "h2T")
                out_sb = actp.tile([1, NCLS], F32, tag="outsb")
                for g0 in range(0, NCLS, 500):
                    gw = min(500, NCLS - g0)
                    ps = psp.tile([128, 512], F32, tag="mm")
                    for c in range(KD):
                        nc.tensor.matmul(ps[0:1, :gw], hT[:, c, :],
                                         whead[:, c, g0:g0 + gw],
                                         start=(c == 0), stop=(c == KD - 1))
                    nc.vector.tensor_copy(out=out_sb[:, g0:g0 + gw],
                                          in_=ps[0:1, :gw])
                nc.sync.dma_start(out=out_d[img:img + 1, :], in_=out_sb)

    nc.finalize()
    return nc


def _prep_inputs(inp):
    bf = ml_dtypes.bfloat16
    f32 = np.float32
    pp = np.asarray(inp["patch_positions"])
    pos_add = (inp["pos_h"][pp[..., 0]] + inp["pos_w"][pp[..., 1]]).astype(f32)

    qg = (inp["qn_g"].reshape(DEPTH, INNER) * SCALE).astype(f32)
    kg = (inp["kn_g"].reshape(DEPTH, INNER) * SCALE).astype(f32)
    pk = (inp["p_kn_g"].reshape(INNER) * SCALE).astype(f32)

    qn = _ln_np(inp["pool_q"].astype(f32), inp["pool_ln_g"]) @ inp["pWq"]
    qn = _rms_np(qn.reshape(HEADS, DHEAD), inp["p_qn_g"]).reshape(INNER)
    qpool = np.zeros((128, HEADS), dtype=bf)
    qpool[0:64, :] = qn.reshape(HEADS, DHEAD).T.astype(bf)
    qpool[64:128, :] = qpool[0:64, :]

    common = dict(
        W_emb=np.asarray(inp["W_emb"]).astype(bf),
        Wq=np.asarray(inp["Wq"]).astype(bf),
        Wkv=np.asarray(inp["Wkv"]).astype(bf),
        Wo=np.asarray(inp["Wo"]).astype(bf),
        W1=np.asarray(inp["W1"]).astype(bf),
        W2=np.asarray(inp["W2"]).astype(bf),
        pWkv=np.asarray(inp["pWkv"]).astype(bf),
        pWo=np.asarray(inp["pWo"]).astype(bf),
        W_head=np.asarray(inp["W_head"]).astype(bf),
        emb_ln_g=np.asarray(inp["emb_ln_g"]).astype(f32),
        emb_ln2_g=np.asarray(inp["emb_ln2_g"]).astype(f32),
        b_emb=np.asarray(inp["b_emb"]).astype(f32),
        ln_attn_g=np.asarray(inp["ln_attn_g"]).astype(f32),
        ln_ff_g=np.asarray(inp["ln_ff_g"]).astype(f32),
        qg_row=qg, kg_row=kg,
        b1=np.asarray(inp["b1"]).astype(f32),
        b2=np.asarray(inp["b2"]).astype(f32),
        final_ln_g=np.asarray(inp["final_ln_g"]).astype(f32),
        pk_row=pk, qpool=qpool,
        pool_q=np.asarray(inp["pool_q"]).astype(f32),
        head_ln_g=np.asarray(inp["head_ln_g"]).astype(f32),
    )
    patches = np.asarray(inp["patches"]).reshape(B * IMGS, TOK_IMG, PATCH_DIM)
    pos_add = pos_add.reshape(B * IMGS, TOK_IMG, DIM)
    in_maps = []
    for c in range(NCORES):
        m = dict(common)
        m["patches"] = np.ascontiguousarray(
            patches[c * 2:(c + 1) * 2].reshape(T, PATCH_DIM)).astype(f32)
        m["pos_add"] = np.ascontiguousarray(
            pos_add[c * 2:(c + 1) * 2].reshape(T, DIM)).astype(f32)
        in_maps.append(m)
    return in_maps


def _is_fast_path(inp):
    ids = np.asarray(inp["image_ids"])
    want = np.broadcast_to(np.repeat(np.arange(IMGS), PH * PW)[None], (B, N))
    return (ids.shape == (B, N) and np.array_equal(ids, want)
            and np.all(np.asarray(inp["lengths"]) == N))


def kernel(**inputs):
    inputs = {k: np.asarray(v) for k, v in inputs.items()}
    if not _is_fast_path(inputs):
        return _reference_np(**inputs)

    from concourse.bass_utils import run_bass_kernel_spmd

    if "nc" not in _CACHE:
        _CACHE["nc"] = build_kernel()
    nc = _CACHE["nc"]
    in_maps = _prep_inputs(inputs)
    res = run_bass_kernel_spmd(nc, in_maps, core_ids=list(range(NCORES)))
    out = np.stack([res.results[c]["out"] for c in range(NCORES)])
    return out.reshape(B, IMGS, NCLS).astype(np.float32)
